# revision 1
# baseline (speedup 1.0000x reference)
"""Trainium2 Bass kernel for nn_Mlp_StaticRoutedLoRAExpert.

Computation (per token chunk with static expert e):
    h = gelu(x @ w1.T + bias1 + SCALE * (x @ a1[e].T) @ b1[e].T)
    y = h @ w2.T + bias2 + SCALE * (h @ a2[e].T) @ b2[e].T

Sharding: data-parallel over batch, 4 batches per core on 8 cores, no
collectives.  Each core computes in feature-major layout (X^T, H^T, Y^T)
so biases live on partitions and the token dim is the matmul moving dim.

Two phases per core (W1^T and W2^T don't fit SBUF together at fp32):
  phase 1: fc1+gelu for all tokens -> H^T scratch in device DRAM
  phase 2: fc2 for all tokens -> Y^T

All matmuls run as float32r (full fp32 storage; relaxed fp32 PE mode,
1 cycle/row at N>=256 - measured ~1.4e-4 rel err vs fp64 reference).
"""

import numpy as np

SCALE = 128.0 / 64.0
B, S, IN, HID, OUT, E, R = 32, 1280, 768, 3072, 768, 2, 64
NCORES = 8
BPC = B // NCORES          # batches per core
TPC = BPC * S              # tokens per core
P = 128
KI = IN // P               # 6  input k-chunks
KH = HID // P              # 24 hidden chunks
KO = OUT // P              # 6  output chunks
MAX_T = 512                # fp32 moving-operand limit

_nc_cache: dict = {}


def _plan_tiles(chunk_sizes, expert_ids):
    """Per-core token tiles: (col_offset, n_tokens, expert)."""
    tiles = []
    for b in range(BPC):
        base = b * S
        start = 0
        for sz, e in zip(chunk_sizes, expert_ids):
            off = 0
            while off < sz:
                t = min(MAX_T, sz - off)
                tiles.append((base + start + off, t, int(e)))
                off += t
            start += sz
    return tuple(tiles)


def _build(tiles, debug_ht=False, timing_internal_io=False, timing_small=False):
    import concourse.bacc as bacc
    import concourse.mybir as mybir
    import concourse.tile as tile

    dt = mybir.dt
    f32 = dt.float32
    f32r = dt.float32r
    AF = mybir.ActivationFunctionType

    nc = bacc.Bacc("TRN2", target_bir_lowering=False, num_devices=NCORES)

    io_kind = "Internal" if timing_internal_io else "ExternalInput"
    out_kind = "Internal" if timing_internal_io else "ExternalOutput"
    tpc = 512 if timing_small else TPC
    xt_d = nc.dram_tensor("xt", [IN, tpc], f32, kind=io_kind)
    w1t_d = nc.dram_tensor("w1t", [IN, HID], f32, kind="ExternalInput")
    b1v_d = nc.dram_tensor("bias1", [HID], f32, kind="ExternalInput")
    a1t_d = nc.dram_tensor("a1t", [E, IN, R], f32, kind="ExternalInput")
    b1t_d = nc.dram_tensor("b1t", [E, R, HID], f32, kind="ExternalInput")
    w2t_d = nc.dram_tensor("w2t", [HID, OUT], f32, kind="ExternalInput")
    b2v_d = nc.dram_tensor("bias2", [OUT], f32, kind="ExternalInput")
    a2t_d = nc.dram_tensor("a2t", [E, HID, R], f32, kind="ExternalInput")
    b2t_d = nc.dram_tensor("b2t", [E, R, OUT], f32, kind="ExternalInput")
    yt_d = nc.dram_tensor("yt", [OUT, tpc], f32, kind=out_kind)
    probe_d = None
    if timing_internal_io:
        probe_d = nc.dram_tensor("probe", [P, KO], f32, kind="ExternalOutput")
    ht_d = nc.dram_tensor("htscr", [HID, tpc], f32,
                          kind="ExternalOutput" if debug_ht else "Internal")

    def rd(ap):
        return ap.bitcast(f32r)

    with tile.TileContext(nc) as tc:
        with tc.tile_pool(name="bias", bufs=1) as bias_pool:
            bias1_s = bias_pool.tile([P, KH], f32)
            nc.sync.dma_start(bias1_s[:], b1v_d.ap().rearrange("(c p) -> p c", p=P))
            bias2_s = bias_pool.tile([P, KO], f32)
            nc.sync.dma_start(bias2_s[:], b2v_d.ap().rearrange("(c p) -> p c", p=P))

            # w2t preloaded during phase 1 (fits alongside phase-1 working set)
            _w2_ctx = tc.tile_pool(name="w2", bufs=1)
            w2_pool = _w2_ctx.__enter__()
            w2t_s = w2_pool.tile([P, KH, OUT], f32r)
            nc.sync.dma_start(
                w2t_s[:], rd(w2t_d.ap().rearrange("(k p) o -> p k o", p=P))
            )

            # ---------------- phase 1: fc1 + gelu ----------------
            with (
                tc.tile_pool(name="w1", bufs=1) as w1_pool,
                tc.tile_pool(name="lora1", bufs=1) as lora1_pool,
                tc.tile_pool(name="xp", bufs=10) as xpool,
                tc.tile_pool(name="hp", bufs=5) as hpool,
                tc.tile_pool(name="u1p", bufs=2) as u1pool,
                tc.tile_pool(name="ps1", bufs=6, space="PSUM") as ps1,
                tc.tile_pool(name="psu1", bufs=2, space="PSUM") as psu1,
            ):
                w1t_s = w1_pool.tile([P, KI, HID], f32r)
                nc.sync.dma_start(
                    w1t_s[:], rd(w1t_d.ap().rearrange("(k p) h -> p k h", p=P))
                )
                a1t_s = lora1_pool.tile([P, E, KI, R], f32r)
                nc.sync.dma_start(
                    a1t_s[:], rd(a1t_d.ap().rearrange("e (k p) r -> p e k r", p=P))
                )
                b1t_s = lora1_pool.tile([R, E, HID], f32r)
                nc.sync.dma_start(b1t_s[:], rd(b1t_d.ap().rearrange("e r h -> r e h")))

                for (col0, T, e) in tiles:
                    col = (col0 % 512 if col0 % 512 + T <= 512 else 0) if timing_small else col0
                    xc = []
                    for k in range(KI):
                        xck = xpool.tile([P, T], f32r, name=f"xc{k}", tag="xc")
                        nc.sync.dma_start(
                            xck[:], rd(xt_d[k * P:(k + 1) * P, col:col + T])
                        )
                        xc.append(xck)
                    u1_ps = psu1.tile([R, T], f32, name="u1ps", tag="u1ps")
                    for k in range(KI):
                        nc.tensor.matmul(
                            u1_ps[:], a1t_s[:, e, k, :], xc[k][:],
                            start=(k == 0), stop=(k == KI - 1),
                        )
                    u1_s = u1pool.tile([R, T], f32r, name="u1s", tag="u1s")
                    nc.vector.tensor_copy(u1_s[:], u1_ps[:])
                    for m in range(KH):
                        h_ps = ps1.tile([P, T], f32, name="hps", tag="hps")
                        for k in range(KI):
                            nc.tensor.matmul(
                                h_ps[:],
                                w1t_s[:, k, m * P:(m + 1) * P],
                                xc[k][:],
                                start=(k == 0), stop=False,
                            )
                        nc.tensor.matmul(
                            h_ps[:],
                            b1t_s[:, e, m * P:(m + 1) * P],
                            u1_s[:],
                            start=False, stop=True,
                        )
                        hc = hpool.tile([P, T], f32r, name="hc", tag="hc")
                        nc.scalar.activation(
                            hc[:], h_ps[:], AF.Gelu, bias=bias1_s[:, m:m + 1]
                        )
                        nc.sync.dma_start(
                            rd(ht_d[m * P:(m + 1) * P, col:col + T]), hc[:]
                        )


            # ---------------- phase 2: fc2 ----------------
            with (
                tc.tile_pool(name="lora2", bufs=1) as lora2_pool,
                tc.tile_pool(name="hp2", bufs=KH + 12) as hpool2,
                tc.tile_pool(name="yp", bufs=8) as ypool,
                tc.tile_pool(name="u2p", bufs=3) as u2pool,
                tc.tile_pool(name="ps2", bufs=6, space="PSUM") as ps2,
                tc.tile_pool(name="psu2", bufs=2, space="PSUM") as psu2,
            ):
                a2t_s = lora2_pool.tile([P, E, KH, R], f32r)
                nc.sync.dma_start(
                    a2t_s[:], rd(a2t_d.ap().rearrange("e (k p) r -> p e k r", p=P))
                )
                b2t_s = lora2_pool.tile([R, E, OUT], f32r)
                nc.sync.dma_start(b2t_s[:], rd(b2t_d.ap().rearrange("e r o -> r e o")))

                for (col0, T, e) in tiles:
                    col = (col0 % 512 if col0 % 512 + T <= 512 else 0) if timing_small else col0
                    hcs = []
                    for m in range(KH):
                        hcm = hpool2.tile([P, T], f32r, name=f"h2_{m}", tag="h2")
                        nc.sync.dma_start(
                            hcm[:], rd(ht_d[m * P:(m + 1) * P, col:col + T])
                        )
                        hcs.append(hcm)
                    u2_ps = psu2.tile([R, T], f32, name="u2ps", tag="u2ps")
                    for m in range(KH):
                        nc.tensor.matmul(
                            u2_ps[:], a2t_s[:, e, m, :], hcs[m][:],
                            start=(m == 0), stop=(m == KH - 1),
                        )
                    u2_s = u2pool.tile([R, T], f32r, name="u2s", tag="u2s")
                    nc.vector.tensor_copy(u2_s[:], u2_ps[:])
                    for o in range(KO):
                        y_ps = ps2.tile([P, T], f32, name="yps", tag="yps")
                        for m in range(KH):
                            nc.tensor.matmul(
                                y_ps[:],
                                w2t_s[:, m, o * P:(o + 1) * P],
                                hcs[m][:],
                                start=(m == 0), stop=False,
                            )
                        nc.tensor.matmul(
                            y_ps[:],
                            b2t_s[:, e, o * P:(o + 1) * P],
                            u2_s[:],
                            start=False, stop=True,
                        )
                        yc = ypool.tile([P, T], f32, name="yc", tag="yc")
                        nc.scalar.activation(
                            yc[:], y_ps[:], AF.Identity, bias=bias2_s[:, o:o + 1]
                        )
                        nc.sync.dma_start(
                            yt_d[o * P:(o + 1) * P, col:col + T], yc[:]
                        )

            _w2_ctx.__exit__(None, None, None)

        if probe_d is not None:
            nc.sync.dma_start(probe_d.ap(), yt_d[0:P, 0:KO])
    nc.compile()
    return nc


def _get_nc(tiles):
    nc = _nc_cache.get(tiles)
    if nc is None:
        nc = _nc_cache[tiles] = _build(tiles)
    return nc


def _run(inputs, trace=False):
    from concourse.bass_utils import run_bass_kernel_spmd

    x = np.asarray(inputs["x"], dtype=np.float32)
    w1 = np.asarray(inputs["w1"], dtype=np.float32)
    bias1 = np.asarray(inputs["bias1"], dtype=np.float32)
    a1 = np.asarray(inputs["a1"], dtype=np.float32)
    b1 = np.asarray(inputs["b1"], dtype=np.float32)
    w2 = np.asarray(inputs["w2"], dtype=np.float32)
    bias2 = np.asarray(inputs["bias2"], dtype=np.float32)
    a2 = np.asarray(inputs["a2"], dtype=np.float32)
    b2 = np.asarray(inputs["b2"], dtype=np.float32)
    chunk_sizes = tuple(int(v) for v in np.asarray(inputs["chunk_sizes"]))
    eids = tuple(int(v) for v in np.asarray(inputs["expert_indices"]))
    assert sum(chunk_sizes) == S

    tiles = _plan_tiles(chunk_sizes, eids)
    nc = _get_nc(tiles)

    xT = np.ascontiguousarray(x.reshape(B * S, IN).T)
    shared = {
        "w1t": np.ascontiguousarray(w1.T),
        "bias1": bias1,
        "a1t": np.ascontiguousarray(a1.transpose(0, 2, 1)),
        "b1t": np.ascontiguousarray((SCALE * b1).transpose(0, 2, 1)),
        "w2t": np.ascontiguousarray(w2.T),
        "bias2": bias2,
        "a2t": np.ascontiguousarray(a2.transpose(0, 2, 1)),
        "b2t": np.ascontiguousarray((SCALE * b2).transpose(0, 2, 1)),
    }
    in_maps = []
    for c in range(NCORES):
        m = dict(shared)
        m["xt"] = np.ascontiguousarray(xT[:, c * TPC:(c + 1) * TPC])
        in_maps.append(m)

    res = run_bass_kernel_spmd(
        nc, in_maps, core_ids=list(range(NCORES)), trace=trace
    )
    yT = np.concatenate([r["yt"] for r in res.results], axis=1)
    y = np.ascontiguousarray(yT.T).reshape(B, S, OUT)
    return y, res


def kernel(**inputs) -> np.ndarray:
    y, _ = _run(inputs, trace=False)
    return y



# revision 2
# speedup vs baseline: 1.2463x; 1.2463x over previous
"""Trainium2 Bass kernel for nn_Mlp_StaticRoutedLoRAExpert.

Computation (per token chunk with static expert e):
    h = gelu(x @ w1.T + bias1 + SCALE * (x @ a1[e].T) @ b1[e].T)
    y = h @ w2.T + bias2 + SCALE * (h @ a2[e].T) @ b2[e].T

Key choices:
  * LoRA is folded on the host into per-expert dense weights
    W1e = w1 + SCALE*b1[e]@a1[e]  (same for layer 2), so the device runs a
    plain two-layer MLP per chunk -- no LoRA matmuls on device.
  * Weights and activations are bf16 (PE runs 1 cycle/row for bf16, same
    as fp32r, but half the SBUF/DMA); accumulation stays fp32 in PSUM.
    Both experts' folded weights fit in SBUF simultaneously.
  * fc1+gelu+fc2 are fused per 512-token tile: the hidden activations
    never leave SBUF (the fp32 two-phase version round-tripped 126MB of
    hidden state through DRAM per core).
  * Data-parallel over batch: 4 batch rows per core on 8 cores, no
    collectives.  Tokens are regrouped per core so every tile is a
    uniform single-expert tile.
"""

import numpy as np
import ml_dtypes

BF16 = ml_dtypes.bfloat16
SCALE = 128.0 / 64.0
B, S, IN, HID, OUT, E, R = 32, 1280, 768, 3072, 768, 2, 64
NCORES = 8
BPC = B // NCORES          # batch rows per core
TPC = BPC * S              # tokens per core
P = 128
KI = IN // P               # 6  input k-chunks
KH = HID // P              # 24 hidden chunks
KO = OUT // P              # 6  output chunks
MAX_T = 512                # PSUM bank limit on the moving dim

_nc_cache: dict = {}


def _plan_tiles(chunk_sizes, expert_ids):
    """Token layout per core: chunks grouped so each tile is one expert.

    Core-local column order: for each input chunk g (in order), the BPC
    batch rows' tokens of that chunk laid out contiguously.  Returns
    (tiles, group_meta) where tiles = ((col, T, expert), ...) and
    group_meta = ((s_start, sz), ...) per chunk for host scatter/gather.
    """
    tiles = []
    groups = []
    col = 0
    s_start = 0
    for sz, e in zip(chunk_sizes, expert_ids):
        groups.append((s_start, sz))
        gsz = BPC * sz
        off = 0
        while off < gsz:
            t = min(MAX_T, gsz - off)
            tiles.append((col + off, t, int(e)))
            off += t
        col += gsz
        s_start += sz
    assert col == TPC
    return tuple(tiles), tuple(groups)


def _build(tiles):
    import concourse.bacc as bacc
    import concourse.mybir as mybir
    import concourse.tile as tile

    dt = mybir.dt
    f32 = dt.float32
    bf16 = dt.bfloat16
    AF = mybir.ActivationFunctionType

    nc = bacc.Bacc("TRN2", target_bir_lowering=False, num_devices=NCORES)

    xt_d = nc.dram_tensor("xt", [IN, TPC], bf16, kind="ExternalInput")
    w1t_d = nc.dram_tensor("w1t", [E, IN, HID], bf16, kind="ExternalInput")
    b1v_d = nc.dram_tensor("bias1", [HID], f32, kind="ExternalInput")
    w2t_d = nc.dram_tensor("w2t", [E, HID, OUT], bf16, kind="ExternalInput")
    b2v_d = nc.dram_tensor("bias2", [OUT], f32, kind="ExternalInput")
    yt_d = nc.dram_tensor("yt", [OUT, TPC], bf16, kind="ExternalOutput")

    with tile.TileContext(nc) as tc:
        with (
            tc.tile_pool(name="const", bufs=1) as cpool,
            tc.tile_pool(name="xp", bufs=12) as xpool,
            tc.tile_pool(name="hp", bufs=28) as hpool,
            tc.tile_pool(name="yp", bufs=6) as ypool,
            tc.tile_pool(name="hps", bufs=6, space="PSUM") as hps_pool,
            tc.tile_pool(name="yps", bufs=2, space="PSUM") as yps_pool,
        ):
            bias1_s = cpool.tile([P, KH], f32)
            nc.sync.dma_start(bias1_s[:], b1v_d.ap().rearrange("(c p) -> p c", p=P))
            bias2_s = cpool.tile([P, KO], f32)
            nc.sync.dma_start(bias2_s[:], b2v_d.ap().rearrange("(c p) -> p c", p=P))
            w1t_s = cpool.tile([P, E, KI, HID], bf16)
            nc.sync.dma_start(
                w1t_s[:], w1t_d.ap().rearrange("e (k p) h -> p e k h", p=P)
            )
            w2t_s = cpool.tile([P, E, KH, OUT], bf16)
            nc.sync.dma_start(
                w2t_s[:], w2t_d.ap().rearrange("e (k p) o -> p e k o", p=P)
            )

            for (col, T, e) in tiles:
                xc = []
                for k in range(KI):
                    xck = xpool.tile([P, T], bf16, name=f"xc{k}", tag="xc")
                    nc.sync.dma_start(xck[:], xt_d[k * P:(k + 1) * P, col:col + T])
                    xc.append(xck)
                hcs = []
                for m in range(KH):
                    h_ps = hps_pool.tile([P, T], f32, name="hps", tag="hps")
                    for k in range(KI):
                        nc.tensor.matmul(
                            h_ps[:],
                            w1t_s[:, e, k, m * P:(m + 1) * P],
                            xc[k][:],
                            start=(k == 0), stop=(k == KI - 1),
                        )
                    hc = hpool.tile([P, T], bf16, name=f"hc{m}", tag="hc")
                    nc.scalar.activation(
                        hc[:], h_ps[:], AF.Gelu, bias=bias1_s[:, m:m + 1]
                    )
                    hcs.append(hc)
                for o in range(KO):
                    y_ps = yps_pool.tile([P, T], f32, name="yps", tag="yps")
                    for m in range(KH):
                        nc.tensor.matmul(
                            y_ps[:],
                            w2t_s[:, e, m, o * P:(o + 1) * P],
                            hcs[m][:],
                            start=(m == 0), stop=(m == KH - 1),
                        )
                    yc = ypool.tile([P, T], bf16, name="yc", tag="yc")
                    nc.scalar.activation(
                        yc[:], y_ps[:], AF.Identity, bias=bias2_s[:, o:o + 1]
                    )
                    nc.sync.dma_start(yt_d[o * P:(o + 1) * P, col:col + T], yc[:])
    nc.compile()
    return nc


def _get_nc(tiles):
    nc = _nc_cache.get(tiles)
    if nc is None:
        nc = _nc_cache[tiles] = _build(tiles)
    return nc


def _run(inputs, trace=False):
    from concourse.bass_utils import run_bass_kernel_spmd

    x = np.asarray(inputs["x"], dtype=np.float32)
    w1 = np.asarray(inputs["w1"], dtype=np.float32)
    bias1 = np.asarray(inputs["bias1"], dtype=np.float32)
    a1 = np.asarray(inputs["a1"], dtype=np.float32)
    b1 = np.asarray(inputs["b1"], dtype=np.float32)
    w2 = np.asarray(inputs["w2"], dtype=np.float32)
    bias2 = np.asarray(inputs["bias2"], dtype=np.float32)
    a2 = np.asarray(inputs["a2"], dtype=np.float32)
    b2 = np.asarray(inputs["b2"], dtype=np.float32)
    chunk_sizes = tuple(int(v) for v in np.asarray(inputs["chunk_sizes"]))
    eids = tuple(int(v) for v in np.asarray(inputs["expert_indices"]))
    assert sum(chunk_sizes) == S

    tiles, groups = _plan_tiles(chunk_sizes, eids)
    nc = _get_nc(tiles)

    # Fold LoRA into per-expert dense weights (host, fp32).
    w1e = w1[None, :, :] + SCALE * np.matmul(b1, a1)     # [E, HID, IN]
    w2e = w2[None, :, :] + SCALE * np.matmul(b2, a2)     # [E, OUT, HID]
    shared = {
        "w1t": np.ascontiguousarray(w1e.transpose(0, 2, 1)).astype(BF16),
        "bias1": bias1,
        "w2t": np.ascontiguousarray(w2e.transpose(0, 2, 1)).astype(BF16),
        "bias2": bias2,
    }

    in_maps = []
    for c in range(NCORES):
        xc = x[c * BPC:(c + 1) * BPC]                    # [BPC, S, IN]
        parts = [
            xc[:, s0:s0 + sz, :].reshape(BPC * sz, IN) for (s0, sz) in groups
        ]
        xt = np.concatenate(parts, axis=0).T             # [IN, TPC]
        m = dict(shared)
        m["xt"] = np.ascontiguousarray(xt).astype(BF16)
        in_maps.append(m)

    res = run_bass_kernel_spmd(
        nc, in_maps, core_ids=list(range(NCORES)), trace=trace
    )

    y = np.empty((B, S, OUT), dtype=np.float32)
    for c in range(NCORES):
        yt = np.asarray(res.results[c]["yt"]).astype(np.float32).T  # [TPC, OUT]
        col = 0
        for (s0, sz) in groups:
            gsz = BPC * sz
            y[c * BPC:(c + 1) * BPC, s0:s0 + sz, :] = (
                yt[col:col + gsz].reshape(BPC, sz, OUT)
            )
            col += gsz
    return y, res


def kernel(**inputs) -> np.ndarray:
    y, _ = _run(inputs, trace=False)
    return y


# revision 4
# speedup vs baseline: 1.3404x; 1.0756x over previous
"""Trainium2 Bass kernel for nn_Mlp_StaticRoutedLoRAExpert.

Computation (per token chunk with static expert e):
    h = gelu(x @ w1.T + bias1 + SCALE * (x @ a1[e].T) @ b1[e].T)
    y = h @ w2.T + bias2 + SCALE * (h @ a2[e].T) @ b2[e].T

Key choices:
  * LoRA is folded on the host into per-expert dense weights
    W1e = w1 + SCALE*b1[e]@a1[e]  (same for layer 2), so the device runs a
    plain two-layer MLP per chunk -- no LoRA matmuls on device.
  * Weights and activations are bf16 (PE runs 1 cycle/row for bf16, same
    as fp32r, but half the SBUF/DMA); accumulation stays fp32 in PSUM.
    Both experts' folded weights fit in SBUF simultaneously (147KB of the
    208KB partition budget).
  * fc1+gelu+fc2 are fused per 512-token tile: the hidden activations
    never leave SBUF (the fp32 two-phase version round-tripped 126MB of
    hidden state through DRAM per core).
  * Data-parallel over batch: 4 batch rows per core on 8 cores, no
    collectives.  Tokens are regrouped per core so every tile is a
    uniform single-expert tile.
  * Weight loads are split into per-128-row chunks issued after the first
    x tile so the PE starts a few us in instead of waiting 57us for two
    monolithic 9.4MB weight DMAs.  x/y move as one DMA per tile.
"""

import numpy as np
import ml_dtypes

BF16 = ml_dtypes.bfloat16
SCALE = 128.0 / 64.0
B, S, IN, HID, OUT, E, R = 32, 1280, 768, 3072, 768, 2, 64
NCORES = 8
BPC = B // NCORES          # batch rows per core
TPC = BPC * S              # tokens per core
P = 128
KI = IN // P               # 6  input k-chunks
KH = HID // P              # 24 hidden chunks
KO = OUT // P              # 6  output chunks
MAX_T = 512                # PSUM bank limit on the moving dim

_nc_cache: dict = {}


def _plan_tiles(chunk_sizes, expert_ids):
    """Token layout per core: chunks grouped so each tile is one expert.

    Core-local column order: for each input chunk g (in order), the BPC
    batch rows' tokens of that chunk laid out contiguously.  Returns
    (tiles, group_meta) where tiles = ((col, T, expert), ...) and
    group_meta = ((s_start, sz), ...) per chunk for host scatter/gather.
    """
    tiles = []
    groups = []
    col = 0
    s_start = 0
    for sz, e in zip(chunk_sizes, expert_ids):
        groups.append((s_start, sz))
        gsz = BPC * sz
        off = 0
        while off < gsz:
            t = min(MAX_T, gsz - off)
            tiles.append((col + off, t, int(e)))
            off += t
        col += gsz
        s_start += sz
    assert col == TPC
    return tuple(tiles), tuple(groups)


def _build(tiles):
    import concourse.bacc as bacc
    import concourse.mybir as mybir
    import concourse.tile as tile

    dt = mybir.dt
    f32 = dt.float32
    bf16 = dt.bfloat16
    AF = mybir.ActivationFunctionType

    nc = bacc.Bacc("TRN2", target_bir_lowering=False, num_devices=NCORES)

    xt_d = nc.dram_tensor("xt", [IN, TPC], bf16, kind="ExternalInput")
    w1t_d = nc.dram_tensor("w1t", [E, IN, HID], bf16, kind="ExternalInput")
    b1v_d = nc.dram_tensor("bias1", [HID], f32, kind="ExternalInput")
    w2t_d = nc.dram_tensor("w2t", [E, HID, OUT], bf16, kind="ExternalInput")
    b2v_d = nc.dram_tensor("bias2", [OUT], f32, kind="ExternalInput")
    yt_d = nc.dram_tensor("yt", [OUT, TPC], bf16, kind="ExternalOutput")

    # experts in first-use order, for weight-load scheduling
    expert_order = []
    for (_, _, e) in tiles:
        if e not in expert_order:
            expert_order.append(e)

    def load_x(xpool, nc, col, T):
        t = xpool.tile([P, KI, T], bf16, name="xio", tag="xio")
        nc.sync.dma_start(
            t[:], xt_d[:, col:col + T].rearrange("(k p) c -> p k c", p=P)
        )
        return t

    with tile.TileContext(nc) as tc:
        with (
            tc.tile_pool(name="const", bufs=1) as cpool,
            tc.tile_pool(name="xp", bufs=3) as xpool,
            tc.tile_pool(name="hp", bufs=25) as hpool,
            tc.tile_pool(name="yp", bufs=2) as ypool,
            tc.tile_pool(name="hps", bufs=6, space="PSUM") as hps_pool,
            tc.tile_pool(name="yps", bufs=2, space="PSUM") as yps_pool,
        ):
            # First x tile before the weight stream so the PE can start
            # as soon as the first w1 chunk lands.
            xio = {0: load_x(xpool, nc, tiles[0][0], tiles[0][1])}

            bias1_s = cpool.tile([P, KH], f32)
            nc.sync.dma_start(bias1_s[:], b1v_d.ap().rearrange("(c p) -> p c", p=P))
            bias2_s = cpool.tile([P, KO], f32)
            nc.sync.dma_start(bias2_s[:], b2v_d.ap().rearrange("(c p) -> p c", p=P))

            # Per-chunk weight tiles, streamed in consumption order.
            w1_s = {}
            w2_s = {}
            for e in expert_order:
                for k in range(KI):
                    t = cpool.tile([P, HID], bf16, name=f"w1_{e}_{k}")
                    nc.sync.dma_start(t[:], w1t_d[e, k * P:(k + 1) * P, :])
                    w1_s[(e, k)] = t
                for m in range(KH):
                    t = cpool.tile([P, OUT], bf16, name=f"w2_{e}_{m}")
                    nc.sync.dma_start(t[:], w2t_d[e, m * P:(m + 1) * P, :])
                    w2_s[(e, m)] = t

            for ti, (col, T, e) in enumerate(tiles):
                if ti not in xio:
                    xio[ti] = load_x(xpool, nc, col, T)
                xcur = xio.pop(ti)
                # prefetch next tile's x
                if ti + 1 < len(tiles) and ti + 1 not in xio:
                    xio[ti + 1] = load_x(xpool, nc, tiles[ti + 1][0],
                                         tiles[ti + 1][1])

                hcs = []
                for m in range(KH):
                    h_ps = hps_pool.tile([P, T], f32, name="hps", tag="hps")
                    for k in range(KI):
                        nc.tensor.matmul(
                            h_ps[:],
                            w1_s[(e, k)][:, m * P:(m + 1) * P],
                            xcur[:, k, :],
                            start=(k == 0), stop=(k == KI - 1),
                        )
                    hc = hpool.tile([P, T], bf16, name=f"hc{m}", tag="hc")
                    nc.scalar.activation(
                        hc[:], h_ps[:], AF.Gelu, bias=bias1_s[:, m:m + 1]
                    )
                    hcs.append(hc)

                yc = ypool.tile([P, KO, T], bf16, name="yio", tag="yio")
                for o in range(KO):
                    y_ps = yps_pool.tile([P, T], f32, name="yps", tag="yps")
                    for m in range(KH):
                        nc.tensor.matmul(
                            y_ps[:],
                            w2_s[(e, m)][:, o * P:(o + 1) * P],
                            hcs[m][:],
                            start=(m == 0), stop=(m == KH - 1),
                        )
                    nc.scalar.activation(
                        yc[:, o, :], y_ps[:], AF.Identity,
                        bias=bias2_s[:, o:o + 1],
                    )
                nc.sync.dma_start(
                    yt_d[:, col:col + T].rearrange("(o p) c -> p o c", p=P),
                    yc[:],
                )
    nc.compile()
    return nc


def _get_nc(tiles):
    nc = _nc_cache.get(tiles)
    if nc is None:
        nc = _nc_cache[tiles] = _build(tiles)
    return nc


def _run(inputs, trace=False):
    from concourse.bass_utils import run_bass_kernel_spmd

    x = np.asarray(inputs["x"], dtype=np.float32)
    w1 = np.asarray(inputs["w1"], dtype=np.float32)
    bias1 = np.asarray(inputs["bias1"], dtype=np.float32)
    a1 = np.asarray(inputs["a1"], dtype=np.float32)
    b1 = np.asarray(inputs["b1"], dtype=np.float32)
    w2 = np.asarray(inputs["w2"], dtype=np.float32)
    bias2 = np.asarray(inputs["bias2"], dtype=np.float32)
    a2 = np.asarray(inputs["a2"], dtype=np.float32)
    b2 = np.asarray(inputs["b2"], dtype=np.float32)
    chunk_sizes = tuple(int(v) for v in np.asarray(inputs["chunk_sizes"]))
    eids = tuple(int(v) for v in np.asarray(inputs["expert_indices"]))
    assert sum(chunk_sizes) == S

    tiles, groups = _plan_tiles(chunk_sizes, eids)
    nc = _get_nc(tiles)

    # Fold LoRA into per-expert dense weights (host, fp32).
    w1e = w1[None, :, :] + SCALE * np.matmul(b1, a1)     # [E, HID, IN]
    w2e = w2[None, :, :] + SCALE * np.matmul(b2, a2)     # [E, OUT, HID]
    shared = {
        "w1t": np.ascontiguousarray(w1e.transpose(0, 2, 1)).astype(BF16),
        "bias1": bias1,
        "w2t": np.ascontiguousarray(w2e.transpose(0, 2, 1)).astype(BF16),
        "bias2": bias2,
    }

    in_maps = []
    for c in range(NCORES):
        xc = x[c * BPC:(c + 1) * BPC]                    # [BPC, S, IN]
        parts = [
            xc[:, s0:s0 + sz, :].reshape(BPC * sz, IN) for (s0, sz) in groups
        ]
        xt = np.concatenate(parts, axis=0).T             # [IN, TPC]
        m = dict(shared)
        m["xt"] = np.ascontiguousarray(xt).astype(BF16)
        in_maps.append(m)

    res = run_bass_kernel_spmd(
        nc, in_maps, core_ids=list(range(NCORES)), trace=trace
    )

    y = np.empty((B, S, OUT), dtype=np.float32)
    for c in range(NCORES):
        yt = np.asarray(res.results[c]["yt"]).astype(np.float32).T  # [TPC, OUT]
        col = 0
        for (s0, sz) in groups:
            gsz = BPC * sz
            y[c * BPC:(c + 1) * BPC, s0:s0 + sz, :] = (
                yt[col:col + gsz].reshape(BPC, sz, OUT)
            )
            col += gsz
    return y, res


def kernel(**inputs) -> np.ndarray:
    y, _ = _run(inputs, trace=False)
    return y
